# revision 1
# baseline (speedup 1.0000x reference)
"""4-layer GCN (EnhancedGCN) on 8 Trainium2 NeuronCores.

Strategy (node/graph parallel):
  - Nodes sharded 12500/core across 8 cores; edges assigned to the core
    owning their dst node.
  - Each layer: h (pre-scaled by norm_src) is replicated across cores via an
    on-device AllGather; each core gathers the src rows for its edges with
    dma_gather (4 SWDGE queues, int16 bank-local indices over 4 x 25000-row
    banks), aggregates them per 128-node dst window with one-hot matmuls on
    the tensor engine (PSUM accumulation over 128-edge subtiles), applies the
    dense W matmul, then norm_dst*z + b fused on the vector engine, GELU
    (+norm_src prescale for the next layer) or the final LayerNorm, and
    writes its output shard.
  - Graph preprocessing (degree norms, edge->core/window/bank grouping,
    padding, gather index layout) happens on host once; the compiled program
    is shared by all 8 cores (SPMD), only the input data differs.
"""

import sys
import types

import numpy as np

N_NODES = 100000
N_EDGES = 1600000
D = 128
NCORES = 8
NPC = N_NODES // NCORES            # 12500 nodes per core
WINDOWS = (NPC + 127) // 128       # 98 dst windows per core (last has 84 rows)
BANKS = 4
BANK_ROWS = N_NODES // BANKS       # 25000 (int16-addressable)
import os as _os
MAX_SUB_PER_GATHER = int(_os.environ.get("KMAXSUB", "8"))  # *128 idx per gather
NQ = int(_os.environ.get("KNQ", "4"))   # SWDGE queues
PAD_DLOC = 999.0

TRACE = False
LAST_EXEC_NS = None

_CACHE = {}


def _install_ntff_hook():
    if "antenv.axon_hooks" in sys.modules:
        return
    mod = types.ModuleType("antenv.axon_hooks")
    _hook = [None]
    mod.set_axon_ntff_profile_hook = lambda h: _hook.__setitem__(0, h)
    mod.get_axon_ntff_profile_hook = lambda: _hook[0]
    sys.modules["antenv.axon_hooks"] = mod
    import antenv

    antenv.axon_hooks = mod
    try:
        from trn_agent_boot.trn_boot import _ntff_profile_via_ctypes

        mod.set_axon_ntff_profile_hook(
            _ntff_profile_via_ctypes("/opt/axon/libaxon_pjrt.so")
        )
    except Exception:
        pass


def _prep_graph(src, dst):
    """Host-side graph preprocessing shared by all layers."""
    src = np.asarray(src).astype(np.int64).ravel()
    dst = np.asarray(dst).astype(np.int64).ravel()

    deg_src = np.bincount(src, minlength=N_NODES).astype(np.float64)
    deg_dst = np.bincount(dst, minlength=N_NODES).astype(np.float64)
    norm_src = np.clip(deg_src, 1.0, None) ** -0.5
    norm_dst = np.clip(deg_dst, 1.0, None) ** -0.5

    core = dst // NPC
    w = (dst % NPC) // 128
    b = src // BANK_ROWS
    group = ((core * WINDOWS + w) * BANKS + b).astype(np.int64)
    order = np.argsort(group, kind="stable")
    g_sorted = group[order]
    src_sorted = src[order]
    dst_sorted = dst[order]

    n_groups = NCORES * WINDOWS * BANKS
    counts = np.bincount(g_sorted, minlength=n_groups).reshape(
        NCORES, WINDOWS, BANKS
    )
    starts = np.zeros(n_groups + 1, np.int64)
    np.cumsum(counts.ravel(), out=starts[1:])

    # shared subtile counts: max over cores, padded to 128
    nsub_shared = np.ceil(counts.max(axis=0) / 128.0).astype(np.int64)  # [W, B]
    nsub_shared = np.maximum(nsub_shared, 1)

    # schedule (same for all cores): per window, list of gather chunks
    schedule = []  # per window: list of (bank, idxcol0, subcol0, nsub_chunk)
    idxcol = 0
    subcol = 0
    for wi in range(WINDOWS):
        gl = []
        for bi in range(BANKS):
            ns = int(nsub_shared[wi, bi])
            off = 0
            while off < ns:
                chunk = min(MAX_SUB_PER_GATHER, ns - off)
                gl.append((bi, idxcol, subcol, chunk))
                idxcol += chunk * 128 // 16
                subcol += chunk
                off += chunk
        schedule.append(gl)
    total_idxcols = idxcol
    total_subs = subcol

    per_core = []
    for c in range(NCORES):
        idx16 = np.zeros((128, total_idxcols), np.int16)
        dloc = np.full((128, total_subs), PAD_DLOC, np.float16)
        for wi in range(WINDOWS):
            by_bank = {}
            for g in schedule[wi]:
                by_bank.setdefault(g[0], []).append(g)
            for bi, chunks in by_bank.items():
                gidx = (c * WINDOWS + wi) * BANKS + bi
                s0, s1 = starts[gidx], starts[gidx + 1]
                e_src = src_sorted[s0:s1]
                e_dst = dst_sorted[s0:s1]
                n_e = s1 - s0
                cap = sum(ch[3] for ch in chunks) * 128
                assert n_e <= cap, (n_e, cap)
                loc = np.zeros(cap, np.int64)
                dl = np.full(cap, PAD_DLOC, np.float64)
                loc[:n_e] = e_src - bi * BANK_ROWS
                dl[:n_e] = (e_dst % NPC) - wi * 128
                off = 0
                for (_, icol0, scol0, chunk) in chunks:
                    nidx = chunk * 128
                    blk_loc = loc[off:off + nidx]
                    blk_dl = dl[off:off + nidx]
                    # idx layout: index i -> partition i%16, col i//16,
                    # replicated across the 8 partition stripes
                    stripe = blk_loc.reshape(nidx // 16, 16).T.astype(np.int16)
                    for s in range(8):
                        idx16[16 * s:16 * s + 16, icol0:icol0 + nidx // 16] = stripe
                    # subtile layout: edge i -> partition i%128, subtile i//128
                    dloc[:, scol0:scol0 + chunk] = (
                        blk_dl.reshape(chunk, 128).T.astype(np.float16)
                    )
                    off += nidx
        onehot = (
            dloc[:, :, None] == np.arange(128, dtype=np.float16)[None, None, :]
        )
        import ml_dtypes
        s8 = onehot.astype(ml_dtypes.float8_e4m3).reshape(128, total_subs * 128)
        per_core.append((idx16, s8))

    def node_tile(vec, c):
        full = np.zeros(WINDOWS * 128, np.float32)
        full[:NPC] = vec[c * NPC:(c + 1) * NPC].astype(np.float32)
        return full.reshape(WINDOWS, 128).T.copy()

    ns_tiles = [node_tile(norm_src, c) for c in range(NCORES)]
    ndn_tiles = [node_tile(norm_dst, c) for c in range(NCORES)]

    return schedule, total_idxcols, total_subs, per_core, ns_tiles, ndn_tiles


def _build_program(schedule, total_idxcols, total_subs):
    import os

    import concourse.bacc as bacc
    import concourse.mybir as mybir
    import concourse.tile as tile

    dbg_layers = int(os.environ.get("DBG_LAYERS", "4"))

    nc = bacc.Bacc(
        "TRN2",
        target_bir_lowering=False,
        debug=False,
        enable_asserts=False,
        num_devices=NCORES,
        num_swdge_queues=NQ,
    )
    f32, f16, i16 = mybir.dt.float32, mybir.dt.float16, mybir.dt.int16
    f8 = mybir.dt.float8e4

    x_in = nc.dram_tensor("x", [NPC, D], f32, kind="ExternalInput")
    idx_in = nc.dram_tensor("idx16", [128, total_idxcols], i16, kind="ExternalInput")
    s8_in = nc.dram_tensor("s8", [128, total_subs * D], f8, kind="ExternalInput")
    ns_in = nc.dram_tensor("ns", [128, WINDOWS], f32, kind="ExternalInput")
    ndn_in = nc.dram_tensor("ndn", [128, WINDOWS], f32, kind="ExternalInput")
    w_in = [nc.dram_tensor(f"W{i+1}", [D, D], f16, kind="ExternalInput") for i in range(4)]
    bb_in = [nc.dram_tensor(f"bb{i+1}", [128, D], f32, kind="ExternalInput") for i in range(4)]
    gam_in = nc.dram_tensor("gamma_b", [128, D], f32, kind="ExternalInput")
    bet_in = nc.dram_tensor("beta_b", [128, D], f32, kind="ExternalInput")
    out = nc.dram_tensor("out", [NPC, D], f32, kind="ExternalOutput")

    Gelu = mybir.ActivationFunctionType.Gelu
    Sqrt = mybir.ActivationFunctionType.Sqrt
    EQ = mybir.AluOpType.is_equal
    MUL = mybir.AluOpType.mult
    SUB = mybir.AluOpType.subtract
    ADD = mybir.AluOpType.add
    X = mybir.AxisListType.X

    qcnt = [0]

    with tile.TileContext(nc) as tc:
        with (
            tc.tile_pool(name="const", bufs=1) as constp,
            tc.tile_pool(name="meta", bufs=1) as metap,
            tc.tile_pool(name="xp", bufs=3) as xp,
            tc.tile_pool(name="msgp", bufs=10) as msgp,
            tc.tile_pool(name="sp", bufs=6) as sp,
            tc.tile_pool(name="aggp", bufs=4) as aggp,
            tc.tile_pool(name="hp", bufs=4) as hp,
            tc.tile_pool(name="lnp", bufs=4) as lnp,
            tc.tile_pool(name="ps1", bufs=3, space="PSUM") as ps1,
            tc.tile_pool(name="ps2", bufs=3, space="PSUM") as ps2,
            tc.tile_pool(name="dram", bufs=1, space="DRAM") as dram,
        ):
            # ---- constants / metadata into SBUF ----
            idx_sb = metap.tile([128, total_idxcols], i16)
            nc.sync.dma_start(idx_sb[:], idx_in[:])
            ns_sb = constp.tile([128, WINDOWS], f32)
            nc.sync.dma_start(ns_sb[:], ns_in[:])
            ndn_sb = constp.tile([128, WINDOWS], f32)
            nc.sync.dma_start(ndn_sb[:], ndn_in[:])
            gam_sb = constp.tile([128, D], f32)
            nc.sync.dma_start(gam_sb[:], gam_in[:])
            bet_sb = constp.tile([128, D], f32)
            nc.sync.dma_start(bet_sb[:], bet_in[:])
            w_sb = []
            bb_sb = []
            for i in range(4):
                wt = constp.tile([D, D], f16, name=f"w{i}_sb")
                nc.sync.dma_start(wt[:], w_in[i][:])
                w_sb.append(wt)
                bt = constp.tile([128, D], f32, name=f"bb{i}_sb")
                nc.sync.dma_start(bt[:], bb_in[i][:])
                bb_sb.append(bt)
            eps_t = constp.tile([128, 1], f32)
            nc.vector.memset(eps_t[:], 1e-5)

            # ---- DRAM h buffers ----
            h_shard = [
                dram.tile([NPC, D], f16, name=f"h_shard{l}") for l in range(4)
            ]
            h_full = [
                dram.tile([N_NODES, D], f16, addr_space="Shared", name=f"h_full{l}")
                for l in range(4)
            ]
            rg = [list(range(NCORES))]

            # ---- prologue: h_shard0 = x * norm_src (cast fp16) ----
            for w in range(WINDOWS):
                rows = min(128, NPC - w * 128)
                xt = xp.tile([128, D], f32, tag="xt")
                nc.sync.dma_start(xt[:rows], x_in[w * 128:w * 128 + rows, :])
                ht = xp.tile([128, D], f16, tag="ht0")
                nc.vector.tensor_scalar(
                    out=ht[:], in0=xt[:], scalar1=ns_sb[:, w:w + 1],
                    scalar2=None, op0=MUL,
                )
                nc.sync.dma_start(h_shard[0][w * 128:w * 128 + rows, :], ht[:rows])
            nc.gpsimd.collective_compute(
                "AllGather", mybir.AluOpType.bypass, replica_groups=rg,
                ins=[h_shard[0][:]], outs=[h_full[0][:]],
            )

            # ---- layers ----
            for l in range(dbg_layers):
                h_in = h_full[l]
                for w in range(WINDOWS):
                    rows = min(128, NPC - w * 128)
                    gathers = schedule[w]
                    n_tot = sum(g[3] for g in gathers)
                    psum1 = ps1.tile([128, 128], f32, tag="psum1")
                    si = 0
                    for (bi, icol0, scol0, chunk) in gathers:
                        nidx = chunk * 128
                        msg = msgp.tile([128, chunk * D], f16, tag="msg")
                        nc.gpsimd.dma_gather(
                            msg[:].rearrange("p (k d) -> p k d", d=D),
                            h_in[bi * BANK_ROWS:(bi + 1) * BANK_ROWS, :],
                            idx_sb[:, icol0:icol0 + nidx // 16],
                            nidx, nidx, D,
                            queue_num=qcnt[0] % NQ,
                        )
                        qcnt[0] += 1
                        # one-hot S slab precomputed on host (fp8, 0/1)
                        s_run = sp.tile([128, chunk * D], f8, tag="s")
                        nc.sync.dma_start(
                            s_run[:],
                            s8_in[:, scol0 * D:(scol0 + chunk) * D],
                        )
                        for s in range(chunk):
                            nc.tensor.matmul(
                                psum1[:],
                                lhsT=msg[:, s * D:(s + 1) * D],
                                rhs=s_run[:, s * D:(s + 1) * D],
                                start=(si == 0), stop=(si == n_tot - 1),
                            )
                            si += 1
                    # dense: z[dst, of] = aggT.T @ W
                    aggT = aggp.tile([128, 128], f16, tag="aggT")
                    nc.scalar.copy(out=aggT[:], in_=psum1[:])
                    psum2 = ps2.tile([128, 128], f32, tag="psum2")
                    nc.tensor.matmul(psum2[:], lhsT=aggT[:], rhs=w_sb[l][:],
                                     start=True, stop=True)
                    # t2 = norm_dst * z + b  (fused on DVE)
                    t2 = hp.tile([128, D], f32, tag="t2")
                    nc.vector.scalar_tensor_tensor(
                        out=t2[:], in0=psum2[:], scalar=ndn_sb[:, w:w + 1],
                        in1=bb_sb[l][:], op0=MUL, op1=ADD,
                    )
                    if l < dbg_layers - 1:
                        g32 = hp.tile([128, D], f32, tag="g32")
                        nc.scalar.activation(out=g32[:], in_=t2[:], func=Gelu)
                        h16 = hp.tile([128, D], f16, tag="h16")
                        nc.vector.tensor_scalar(
                            out=h16[:], in0=g32[:],
                            scalar1=ns_sb[:, w:w + 1], scalar2=None, op0=MUL,
                        )
                        nc.sync.dma_start(
                            h_shard[l + 1][w * 128:w * 128 + rows, :], h16[:rows]
                        )
                    else:
                        # LayerNorm over features
                        s1 = lnp.tile([128, 1], f32, tag="s1")
                        nc.vector.reduce_sum(s1[:], t2[:], axis=X)
                        mu = lnp.tile([128, 1], f32, tag="mu")
                        nc.scalar.mul(out=mu[:], in_=s1[:], mul=1.0 / D)
                        cent = lnp.tile([128, D], f32, tag="cent")
                        nc.vector.tensor_scalar(
                            out=cent[:], in0=t2[:], scalar1=mu[:],
                            scalar2=None, op0=SUB,
                        )
                        sq = lnp.tile([128, D], f32, tag="sq")
                        nc.vector.tensor_tensor(out=sq[:], in0=cent[:],
                                                in1=cent[:], op=MUL)
                        vs = lnp.tile([128, 1], f32, tag="vs")
                        nc.vector.reduce_sum(vs[:], sq[:], axis=X)
                        std = lnp.tile([128, 1], f32, tag="std")
                        nc.scalar.activation(out=std[:], in_=vs[:], func=Sqrt,
                                             scale=1.0 / D, bias=eps_t[:])
                        rstd = lnp.tile([128, 1], f32, tag="rstd")
                        nc.vector.reciprocal(out=rstd[:], in_=std[:])
                        t1 = lnp.tile([128, D], f32, tag="t1")
                        nc.vector.tensor_scalar(out=t1[:], in0=cent[:],
                                                scalar1=rstd[:], scalar2=None,
                                                op0=MUL)
                        t4 = lnp.tile([128, D], f32, tag="t4")
                        nc.vector.tensor_tensor(out=t4[:], in0=t1[:],
                                                in1=gam_sb[:], op=MUL)
                        t5 = lnp.tile([128, D], f32, tag="t5")
                        nc.vector.tensor_tensor(out=t5[:], in0=t4[:],
                                                in1=bet_sb[:], op=ADD)
                        nc.sync.dma_start(
                            out[w * 128:w * 128 + rows, :], t5[:rows]
                        )
                if l < dbg_layers - 1:
                    nc.gpsimd.collective_compute(
                        "AllGather", mybir.AluOpType.bypass, replica_groups=rg,
                        ins=[h_shard[l + 1][:]], outs=[h_full[l + 1][:]],
                    )
    nc.compile()
    return nc


def kernel(**inputs):
    global LAST_EXEC_NS
    from concourse.bass_utils import run_bass_kernel_spmd

    x = np.asarray(inputs["x"], np.float32)
    src = inputs["src"]
    dst = inputs["dst"]

    key = "prog"
    if key not in _CACHE:
        schedule, tic, tsc, per_core, ns_tiles, ndn_tiles = _prep_graph(src, dst)
        nc = _build_program(schedule, tic, tsc)
        _CACHE[key] = (nc, per_core, ns_tiles, ndn_tiles)
    nc, per_core, ns_tiles, ndn_tiles = _CACHE[key]

    gamma = np.asarray(inputs["gamma"], np.float32).reshape(1, D)
    beta = np.asarray(inputs["beta"], np.float32).reshape(1, D)
    gamma_b = np.repeat(gamma, 128, axis=0)
    beta_b = np.repeat(beta, 128, axis=0)

    in_maps = []
    for c in range(NCORES):
        idx16, s8 = per_core[c]
        m = {
            "x": np.ascontiguousarray(x[c * NPC:(c + 1) * NPC]),
            "idx16": idx16,
            "s8": s8,
            "ns": ns_tiles[c],
            "ndn": ndn_tiles[c],
            "gamma_b": gamma_b,
            "beta_b": beta_b,
        }
        for i in range(4):
            m[f"W{i+1}"] = np.asarray(inputs[f"W{i+1}"], np.float32).astype(np.float16)
            bb = np.asarray(inputs[f"b{i+1}"], np.float32).reshape(1, D)
            m[f"bb{i+1}"] = np.repeat(bb, 128, axis=0)
        in_maps.append(m)

    if TRACE:
        _install_ntff_hook()
    res = run_bass_kernel_spmd(
        nc, in_maps, core_ids=list(range(NCORES)), trace=TRACE
    )
    LAST_EXEC_NS = res.exec_time_ns
    return np.concatenate(
        [res.results[c]["out"] for c in range(NCORES)], axis=0
    ).astype(np.float32)



# revision 10
# speedup vs baseline: 1.4132x; 1.4132x over previous
"""4-layer GCN (EnhancedGCN) on 8 Trainium2 NeuronCores.

Strategy (node/graph parallel, v2):
  - Nodes sharded 12500/core across 8 cores; edges assigned to the core
    owning their dst node.
  - Aggregation granularity is a "window group" of 4 dst windows (512 dst
    slots, one PSUM bank worth) per src bank: edges sorted by
    (core, wgroup, bank, dst) so each (wgroup, bank) needs ONE dma_gather
    (~16 subtiles of 128 edges) instead of 4 -- 4x fewer gpsimd descriptor
    ops and ~20% less ceil-to-128 padding in gather traffic.
  - One-hot scatter matmuls accumulate each 128-edge subtile into the
    quadrant PSUM tiles its (sorted) dst rows touch; the quadrant union is
    taken across cores so the SPMD program is shared.
  - Per 128-dst window: psum1 -> aggT (scalar copy, fp16), dense W matmul
    plus a rank-1 bias matmul with lhsT = 1/norm_dst row (so the bias
    survives the later norm_dst scale), then Gelu(scale=norm_dst) and
    Copy(scale=norm_src) on the SCALAR engine -- the DVE is kept out of the
    steady state entirely (it shares SBUF ports with gpsimd and stalled
    5-17us per op in the baseline).
  - Final layer: LayerNorm with scalar-engine accum_out row sums; only the
    reciprocal and gamma/beta tensor ops run on the DVE.
  - Graph preprocessing (degree norms, edge->group/bank sort, padding,
    gather index layout, one-hot slabs) happens on host once; the compiled
    program is shared by all 8 cores (SPMD), only the input data differs.
"""

import sys
import types

import numpy as np

N_NODES = 100000
N_EDGES = 1600000
D = 128
NCORES = 8
NPC = N_NODES // NCORES            # 12500 nodes per core
WINDOWS = (NPC + 127) // 128       # 98 dst windows per core (last has 84 rows)
WG_WIN = 4                         # windows per PSUM group (512 dst slots)
NWG = (WINDOWS + WG_WIN - 1) // WG_WIN   # 25 window groups (last has 2 windows)
BANKS = 4
BANK_ROWS = N_NODES // BANKS       # 25000 (int16-addressable)
import os as _os
MAXSUB = int(_os.environ.get("KMAXSUB", "8"))  # max 128-edge subtiles per dma_gather
NQ = 4                             # SWDGE queues (hw max)

TRACE = False
LAST_EXEC_NS = None

_CACHE = {}


def _install_ntff_hook():
    if "antenv.axon_hooks" in sys.modules:
        return
    mod = types.ModuleType("antenv.axon_hooks")
    _hook = [None]
    mod.set_axon_ntff_profile_hook = lambda h: _hook.__setitem__(0, h)
    mod.get_axon_ntff_profile_hook = lambda: _hook[0]
    sys.modules["antenv.axon_hooks"] = mod
    import antenv

    antenv.axon_hooks = mod
    try:
        from trn_agent_boot.trn_boot import _ntff_profile_via_ctypes

        mod.set_axon_ntff_profile_hook(
            _ntff_profile_via_ctypes("/opt/axon/libaxon_pjrt.so")
        )
    except Exception:
        pass


def _prep_graph(src, dst):
    """Host-side graph preprocessing shared by all layers."""
    import ml_dtypes

    src = np.asarray(src).astype(np.int64).ravel()
    dst = np.asarray(dst).astype(np.int64).ravel()

    deg_src = np.bincount(src, minlength=N_NODES).astype(np.float64)
    deg_dst = np.bincount(dst, minlength=N_NODES).astype(np.float64)
    norm_src = np.clip(deg_src, 1.0, None) ** -0.5
    norm_dst = np.clip(deg_dst, 1.0, None) ** -0.5
    inv_norm_dst = np.sqrt(np.clip(deg_dst, 1.0, None))

    core = dst // NPC
    j = dst % NPC
    wg = (j // 128) // WG_WIN
    b = src // BANK_ROWS
    blk = (core * NWG + wg) * BANKS + b
    key = (blk.astype(np.int64) << 20) | j
    order = np.argsort(key, kind="stable")
    src_s = src[order]
    j_s = j[order]

    n_blk = NCORES * NWG * BANKS
    counts = np.bincount(blk[order], minlength=n_blk).reshape(NCORES, NWG, BANKS)
    starts = np.zeros(n_blk + 1, np.int64)
    np.cumsum(counts.ravel(), out=starts[1:])
    nsub = np.maximum(1, -(-counts.max(axis=0) // 128))  # [NWG, BANKS]

    # per-core padded dloc/loc blocks; dloc pad = -1, loc pad = 0 (row 0 is
    # real finite data so padded gathers cannot inject NaN into the matmul)
    dloc_blocks = {}
    loc_blocks = {}
    for c in range(NCORES):
        for g in range(NWG):
            for bi in range(BANKS):
                gi = (c * NWG + g) * BANKS + bi
                s0, s1 = starts[gi], starts[gi + 1]
                cap = int(nsub[g, bi]) * 128
                dl = np.full(cap, -1, np.int64)
                lo = np.zeros(cap, np.int64)
                n_e = s1 - s0
                assert n_e <= cap, (n_e, cap)
                dl[:n_e] = j_s[s0:s1] - g * (WG_WIN * 128)
                lo[:n_e] = src_s[s0:s1] - bi * BANK_ROWS
                dloc_blocks[(c, g, bi)] = dl
                loc_blocks[(c, g, bi)] = lo

    # shared schedule: per wg, gathers (one per bank, chunked at MAXSUB) and
    # the (subtile, quadrant) matmul pairs with start/stop flags
    sched = []
    icol = 0
    pcol = 0
    for g in range(NWG):
        nwin = min(WG_WIN, WINDOWS - g * WG_WIN)
        bank_pairs = []
        for bi in range(BANKS):
            ns_ = int(nsub[g, bi])
            qsets = [set() for _ in range(ns_)]
            for c in range(NCORES):
                dl = dloc_blocks[(c, g, bi)]
                for s in range(ns_):
                    rows = dl[s * 128:(s + 1) * 128]
                    qs = np.unique(rows[rows >= 0] // 128)
                    qsets[s].update(int(x) for x in qs)
            bank_pairs.append(
                [(s, q) for s in range(ns_) for q in sorted(qsets[s])]
            )
        covered = set(q for bp in bank_pairs for (_, q) in bp)
        for q in range(nwin):
            if q not in covered:
                bank_pairs[0].insert(0, (0, q))
        all_flat = [(bi, s, q) for bi in range(BANKS) for (s, q) in bank_pairs[bi]]
        firsts = {}
        lasts = {}
        for i, (_, _, q) in enumerate(all_flat):
            firsts.setdefault(q, i)
            lasts[q] = i
        gathers = []
        i_flat = 0
        for bi in range(BANKS):
            ns_ = int(nsub[g, bi])
            pl = bank_pairs[bi]
            off = 0
            while off < ns_:
                csub = min(MAXSUB, ns_ - off)
                mm = []
                for (s, q) in pl:
                    if off <= s < off + csub:
                        mm.append((s - off, pcol, q,
                                   firsts[q] == i_flat, lasts[q] == i_flat))
                        pcol += 1
                        i_flat += 1
                nidx = csub * 128
                gathers.append(dict(b=bi, icol0=icol, nidx=nidx, mm=mm,
                                    sub0=off))
                icol += nidx // 16
                off += csub
        sched.append(dict(nwin=nwin, gathers=gathers))
    total_idxcols = icol
    total_pairs = pcol

    # per-core gather indices + one-hot slabs
    ar128 = np.arange(128)
    per_core = []
    for c in range(NCORES):
        idx16 = np.zeros((128, total_idxcols), np.int16)
        s8 = np.zeros((128, total_pairs * 128), ml_dtypes.float8_e4m3)
        for g in range(NWG):
            for gt in sched[g]["gathers"]:
                bi, icol0, nidx, sub0 = gt["b"], gt["icol0"], gt["nidx"], gt["sub0"]
                lo = loc_blocks[(c, g, bi)][sub0 * 128: sub0 * 128 + nidx]
                stripe = lo.reshape(nidx // 16, 16).T.astype(np.int16)
                for st in range(8):
                    idx16[16 * st:16 * st + 16, icol0:icol0 + nidx // 16] = stripe
                dl = dloc_blocks[(c, g, bi)]
                for (s_loc, pc_, q, _, _) in gt["mm"]:
                    rows = dl[(sub0 + s_loc) * 128:(sub0 + s_loc + 1) * 128]
                    rel = rows - q * 128
                    valid = (rel >= 0) & (rel < 128)
                    m = (rel[:, None] == ar128[None, :]) & valid[:, None]
                    s8[:, pc_ * 128:(pc_ + 1) * 128] = m.astype(
                        ml_dtypes.float8_e4m3
                    )
        per_core.append((idx16, s8))

    def node_tile(vec, c):
        full = np.zeros(WINDOWS * 128, np.float32)
        full[:NPC] = vec[c * NPC:(c + 1) * NPC].astype(np.float32)
        return full.reshape(WINDOWS, 128).T.copy()

    ns_tiles = [node_tile(norm_src, c) for c in range(NCORES)]
    ndn_tiles = [node_tile(norm_dst, c) for c in range(NCORES)]
    invndn_tiles = []
    for c in range(NCORES):
        t = np.zeros((16, WINDOWS * 128), np.float16)
        t[0, :NPC] = inv_norm_dst[c * NPC:(c + 1) * NPC].astype(np.float16)
        invndn_tiles.append(t)

    return sched, total_idxcols, total_pairs, per_core, ns_tiles, ndn_tiles, invndn_tiles


def _build_program(sched, total_idxcols, total_pairs):
    import os

    import concourse.bacc as bacc
    import concourse.mybir as mybir
    import concourse.tile as tile

    dbg_layers = int(os.environ.get("DBG_LAYERS", "4"))

    nc = bacc.Bacc(
        "TRN2",
        target_bir_lowering=False,
        debug=False,
        enable_asserts=False,
        num_devices=NCORES,
        num_swdge_queues=NQ,
    )
    f32, f16, i16 = mybir.dt.float32, mybir.dt.float16, mybir.dt.int16
    f8 = mybir.dt.float8e4

    x_in = nc.dram_tensor("x", [NPC, D], f32, kind="ExternalInput")
    idx_in = nc.dram_tensor("idx16", [128, total_idxcols], i16, kind="ExternalInput")
    s8_in = nc.dram_tensor("s8", [128, total_pairs * D], f8, kind="ExternalInput")
    ns_in = nc.dram_tensor("ns", [128, WINDOWS], f32, kind="ExternalInput")
    ndn_in = nc.dram_tensor("ndn", [128, WINDOWS], f32, kind="ExternalInput")
    invndn_in = nc.dram_tensor("invndn", [16, WINDOWS * 128], f16, kind="ExternalInput")
    w_in = [nc.dram_tensor(f"W{i+1}", [D, D], f16, kind="ExternalInput") for i in range(4)]
    brow_in = [nc.dram_tensor(f"brow{i+1}", [16, D], f16, kind="ExternalInput") for i in range(4)]
    gam_in = nc.dram_tensor("gamma_b", [128, D], f32, kind="ExternalInput")
    bet_in = nc.dram_tensor("beta_b", [128, D], f32, kind="ExternalInput")
    out = nc.dram_tensor("out", [NPC, D], f32, kind="ExternalOutput")

    Gelu = mybir.ActivationFunctionType.Gelu
    Sqrt = mybir.ActivationFunctionType.Sqrt
    Copy = mybir.ActivationFunctionType.Copy
    Ident = mybir.ActivationFunctionType.Identity
    Square = mybir.ActivationFunctionType.Square
    MUL = mybir.AluOpType.mult
    ADD = mybir.AluOpType.add

    qcnt = [0]

    with tile.TileContext(nc) as tc:
        with (
            tc.tile_pool(name="const", bufs=1) as constp,
            tc.tile_pool(name="meta", bufs=1) as metap,
            tc.tile_pool(name="xp", bufs=3) as xp,
            tc.tile_pool(name="msgp", bufs=24) as msgp,
            tc.tile_pool(name="sp", bufs=24) as sp,
            tc.tile_pool(name="aggp", bufs=6) as aggp,
            tc.tile_pool(name="hp", bufs=6) as hp,
            tc.tile_pool(name="lnp", bufs=4) as lnp,
            tc.tile_pool(name="ps1", bufs=3, space="PSUM") as ps1,
            tc.tile_pool(name="ps2", bufs=4, space="PSUM") as ps2,
            tc.tile_pool(name="dram", bufs=1, space="DRAM") as dram,
        ):
            # ---- constants / metadata into SBUF ----
            idx_sb = metap.tile([128, total_idxcols], i16)
            nc.sync.dma_start(idx_sb[:], idx_in[:])
            ns_sb = constp.tile([128, WINDOWS], f32)
            nc.sync.dma_start(ns_sb[:], ns_in[:])
            ndn_sb = constp.tile([128, WINDOWS], f32)
            nc.sync.dma_start(ndn_sb[:], ndn_in[:])
            invndn_sb = constp.tile([16, WINDOWS * 128], f16)
            nc.sync.dma_start(invndn_sb[:], invndn_in[:])
            gam_sb = constp.tile([128, D], f32)
            nc.sync.dma_start(gam_sb[:], gam_in[:])
            bet_sb = constp.tile([128, D], f32)
            nc.sync.dma_start(bet_sb[:], bet_in[:])
            w_sb = []
            brow_sb = []
            for i in range(4):
                wt = constp.tile([D, D], f16, name=f"w{i}_sb")
                nc.sync.dma_start(wt[:], w_in[i][:])
                w_sb.append(wt)
                bt = constp.tile([16, D], f16, name=f"brow{i}_sb")
                nc.sync.dma_start(bt[:], brow_in[i][:])
                brow_sb.append(bt)
            eps_t = constp.tile([128, 1], f32)
            nc.vector.memset(eps_t[:], 1e-5)

            # ---- DRAM h buffers ----
            h_shard = [
                dram.tile([NPC, D], f16, name=f"h_shard{l}") for l in range(4)
            ]
            h_full = [
                dram.tile([N_NODES, D], f16, addr_space="Shared", name=f"h_full{l}")
                for l in range(4)
            ]
            rg = [list(range(NCORES))]

            # ---- prologue: h_shard0 = x * norm_src (cast fp16) ----
            for w in range(WINDOWS):
                rows = min(128, NPC - w * 128)
                xt = xp.tile([128, D], f32, tag="xt")
                nc.sync.dma_start(xt[:rows], x_in[w * 128:w * 128 + rows, :])
                ht = xp.tile([128, D], f16, tag="ht0")
                nc.scalar.activation(out=ht[:], in_=xt[:], func=Copy,
                                     scale=ns_sb[:, w:w + 1])
                nc.sync.dma_start(h_shard[0][w * 128:w * 128 + rows, :], ht[:rows])
            nc.gpsimd.collective_compute(
                "AllGather", mybir.AluOpType.bypass, replica_groups=rg,
                ins=[h_shard[0][:]], outs=[h_full[0][:]],
            )

            # ---- layers ----
            for l in range(dbg_layers):
                h_in = h_full[l]
                for g in range(NWG):
                    nwin = sched[g]["nwin"]
                    psg = ps1.tile([128, WG_WIN * 128], f32, tag="psg")
                    qpairs = [[] for _ in range(nwin)]
                    for gt in sched[g]["gathers"]:
                        bi, icol0, nidx = gt["b"], gt["icol0"], gt["nidx"]
                        mm = gt["mm"]
                        if not mm:
                            continue
                        csub = nidx // 128
                        msg = msgp.tile([128, csub * D], f16, tag="msg")
                        nc.gpsimd.dma_gather(
                            msg[:].rearrange("p (k d) -> p k d", d=D),
                            h_in[bi * BANK_ROWS:(bi + 1) * BANK_ROWS, :],
                            idx_sb[:, icol0:icol0 + nidx // 16],
                            nidx, nidx, D,
                            queue_num=qcnt[0] % NQ,
                        )
                        qcnt[0] += 1
                        np_ = len(mm)
                        pc0 = mm[0][1]
                        s_run = sp.tile([128, np_ * D], f8, tag="s")
                        nc.sync.dma_start(
                            s_run[:],
                            s8_in[:, pc0 * D:(pc0 + np_) * D],
                        )
                        for (s_loc, pc_, q, _, _) in mm:
                            qpairs[q].append((msg, s_run, pc0, s_loc, pc_))
                    # contiguous start->stop accumulation group per quadrant
                    for q in range(nwin):
                        n_q = len(qpairs[q])
                        for i, (msg, s_run, pc0, s_loc, pc_) in enumerate(
                                qpairs[q]):
                            nc.tensor.matmul(
                                psg[:, q * 128:(q + 1) * 128],
                                lhsT=msg[:, s_loc * D:(s_loc + 1) * D],
                                rhs=s_run[:, (pc_ - pc0) * D:(pc_ - pc0 + 1) * D],
                                start=(i == 0), stop=(i == n_q - 1),
                            )
                    for qi in range(nwin):
                        w = g * WG_WIN + qi
                        rows = min(128, NPC - w * 128)
                        aggT = aggp.tile([128, 128], f16, tag="aggT")
                        nc.scalar.copy(out=aggT[:],
                                       in_=psg[:, qi * 128:(qi + 1) * 128])
                        psum2 = ps2.tile([128, 128], f32, tag="psum2")
                        nc.tensor.matmul(psum2[:], lhsT=aggT[:], rhs=w_sb[l][:],
                                         start=True, stop=False)
                        nc.tensor.matmul(
                            psum2[:],
                            lhsT=invndn_sb[:, w * 128:(w + 1) * 128],
                            rhs=brow_sb[l][:],
                            start=False, stop=True,
                        )
                        if l < dbg_layers - 1:
                            g32 = hp.tile([128, D], f32, tag="g32")
                            nc.scalar.activation(out=g32[:], in_=psum2[:],
                                                 func=Gelu,
                                                 scale=ndn_sb[:, w:w + 1])
                            h16 = hp.tile([128, D], f16, tag="h16")
                            nc.scalar.activation(out=h16[:], in_=g32[:],
                                                 func=Copy,
                                                 scale=ns_sb[:, w:w + 1])
                            nc.sync.dma_start(
                                h_shard[l + 1][w * 128:w * 128 + rows, :],
                                h16[:rows],
                            )
                        else:
                            # LayerNorm over features (scalar engine + accum)
                            t2 = lnp.tile([128, D], f32, tag="t2")
                            s1 = lnp.tile([128, 1], f32, tag="s1")
                            nc.scalar.activation(out=t2[:], in_=psum2[:],
                                                 func=Copy,
                                                 scale=ndn_sb[:, w:w + 1],
                                                 accum_out=s1[:])
                            negmu = lnp.tile([128, 1], f32, tag="negmu")
                            nc.scalar.mul(out=negmu[:], in_=s1[:], mul=-1.0 / D)
                            cent = lnp.tile([128, D], f32, tag="cent")
                            nc.scalar.activation(out=cent[:], in_=t2[:],
                                                 func=Ident, bias=negmu[:])
                            sq = lnp.tile([128, D], f32, tag="sq")
                            vs = lnp.tile([128, 1], f32, tag="vs")
                            nc.scalar.activation(out=sq[:], in_=cent[:],
                                                 func=Square, accum_out=vs[:])
                            std = lnp.tile([128, 1], f32, tag="std")
                            nc.scalar.activation(out=std[:], in_=vs[:],
                                                 func=Sqrt, scale=1.0 / D,
                                                 bias=eps_t[:])
                            rstd = lnp.tile([128, 1], f32, tag="rstd")
                            nc.vector.reciprocal(out=rstd[:], in_=std[:])
                            t1 = lnp.tile([128, D], f32, tag="t1")
                            nc.scalar.activation(out=t1[:], in_=cent[:],
                                                 func=Copy, scale=rstd[:])
                            t4 = lnp.tile([128, D], f32, tag="t4")
                            nc.vector.tensor_tensor(out=t4[:], in0=t1[:],
                                                    in1=gam_sb[:], op=MUL)
                            t5 = lnp.tile([128, D], f32, tag="t5")
                            nc.vector.tensor_tensor(out=t5[:], in0=t4[:],
                                                    in1=bet_sb[:], op=ADD)
                            nc.sync.dma_start(
                                out[w * 128:w * 128 + rows, :], t5[:rows]
                            )
                if l < dbg_layers - 1:
                    nc.gpsimd.collective_compute(
                        "AllGather", mybir.AluOpType.bypass, replica_groups=rg,
                        ins=[h_shard[l + 1][:]], outs=[h_full[l + 1][:]],
                    )
    nc.compile()
    return nc


def kernel(**inputs):
    global LAST_EXEC_NS
    from concourse.bass_utils import run_bass_kernel_spmd

    x = np.asarray(inputs["x"], np.float32)
    src = inputs["src"]
    dst = inputs["dst"]

    key = "prog"
    if key not in _CACHE:
        sched, tic, tpc, per_core, ns_tiles, ndn_tiles, invndn_tiles = _prep_graph(src, dst)
        nc = _build_program(sched, tic, tpc)
        _CACHE[key] = (nc, per_core, ns_tiles, ndn_tiles, invndn_tiles)
    nc, per_core, ns_tiles, ndn_tiles, invndn_tiles = _CACHE[key]

    gamma = np.asarray(inputs["gamma"], np.float32).reshape(1, D)
    beta = np.asarray(inputs["beta"], np.float32).reshape(1, D)
    gamma_b = np.repeat(gamma, 128, axis=0)
    beta_b = np.repeat(beta, 128, axis=0)

    in_maps = []
    for c in range(NCORES):
        idx16, s8 = per_core[c]
        m = {
            "x": np.ascontiguousarray(x[c * NPC:(c + 1) * NPC]),
            "idx16": idx16,
            "s8": s8,
            "ns": ns_tiles[c],
            "ndn": ndn_tiles[c],
            "invndn": invndn_tiles[c],
            "gamma_b": gamma_b,
            "beta_b": beta_b,
        }
        for i in range(4):
            m[f"W{i+1}"] = np.asarray(inputs[f"W{i+1}"], np.float32).astype(np.float16)
            br = np.zeros((16, D), np.float16)
            br[0] = np.asarray(inputs[f"b{i+1}"], np.float32).astype(np.float16)
            m[f"brow{i+1}"] = br
        in_maps.append(m)

    if TRACE:
        _install_ntff_hook()
    res = run_bass_kernel_spmd(
        nc, in_maps, core_ids=list(range(NCORES)), trace=TRACE
    )
    LAST_EXEC_NS = res.exec_time_ns
    return np.concatenate(
        [res.results[c]["out"] for c in range(NCORES)], axis=0
    ).astype(np.float32)


# revision 19
# speedup vs baseline: 1.4624x; 1.0349x over previous
"""4-layer GCN (EnhancedGCN) on 8 Trainium2 NeuronCores.

Strategy (node/graph parallel, v2):
  - Nodes sharded 12500/core across 8 cores; edges assigned to the core
    owning their dst node.
  - Aggregation granularity is a "window group" of 4 dst windows (512 dst
    slots, one PSUM bank worth) per src bank: edges sorted by
    (core, wgroup, bank, dst) so each (wgroup, bank) needs ONE dma_gather
    (~16 subtiles of 128 edges) instead of 4 -- 4x fewer gpsimd descriptor
    ops and ~20% less ceil-to-128 padding in gather traffic.
  - One-hot scatter matmuls accumulate each 128-edge subtile into the
    quadrant PSUM tiles its (sorted) dst rows touch; the quadrant union is
    taken across cores so the SPMD program is shared.
  - Per 128-dst window: psum1 -> aggT (scalar copy, fp16), dense W matmul
    plus a rank-1 bias matmul with lhsT = 1/norm_dst row (so the bias
    survives the later norm_dst scale), then Gelu(scale=norm_dst) and
    Copy(scale=norm_src) on the SCALAR engine -- the DVE is kept out of the
    steady state entirely (it shares SBUF ports with gpsimd and stalled
    5-17us per op in the baseline).
  - Final layer: LayerNorm with scalar-engine accum_out row sums; only the
    reciprocal and gamma/beta tensor ops run on the DVE.
  - Graph preprocessing (degree norms, edge->group/bank sort, padding,
    gather index layout, one-hot slabs) happens on host once; the compiled
    program is shared by all 8 cores (SPMD), only the input data differs.
"""

import sys
import types

import numpy as np

N_NODES = 100000
N_EDGES = 1600000
D = 128
NCORES = 8
NPC = N_NODES // NCORES            # 12500 nodes per core
WINDOWS = (NPC + 127) // 128       # 98 dst windows per core (last has 84 rows)
WG_WIN = 4                         # windows per PSUM group (512 dst slots)
NWG = (WINDOWS + WG_WIN - 1) // WG_WIN   # 25 window groups (last has 2 windows)
BANKS = 4
# h is exchanged in two half-shard AllGathers so the first collective
# overlaps the second half's compute. Half A = windows 0..47 (6144
# rows/core), half B = windows 48..97 (6356 rows/core).
HALF_A_ROWS = 48 * 128             # 6144 per core
HALF_B_ROWS = NPC - HALF_A_ROWS    # 6356 per core
HA_TOT = NCORES * HALF_A_ROWS      # 49152
HB_TOT = NCORES * HALF_B_ROWS      # 50848
# bank -> (half, row offset, rows); all < 32768 so int16 indices work
BANK_DEFS = (
    (0, 0, HA_TOT - HA_TOT // 2), (0, HA_TOT - HA_TOT // 2, HA_TOT // 2),
    (1, 0, HB_TOT - HB_TOT // 2), (1, HB_TOT - HB_TOT // 2, HB_TOT // 2),
)
WG_SPLIT = 12                      # collective A fires after wg 0..11 (windows 0..47)
import os as _os
MAXSUB = int(_os.environ.get("KMAXSUB", "8"))  # max 128-edge subtiles per dma_gather
NQ = 4                             # SWDGE queues (hw max)

TRACE = False
LAST_EXEC_NS = None

_CACHE = {}


def _install_ntff_hook():
    if "antenv.axon_hooks" in sys.modules:
        return
    mod = types.ModuleType("antenv.axon_hooks")
    _hook = [None]
    mod.set_axon_ntff_profile_hook = lambda h: _hook.__setitem__(0, h)
    mod.get_axon_ntff_profile_hook = lambda: _hook[0]
    sys.modules["antenv.axon_hooks"] = mod
    import antenv

    antenv.axon_hooks = mod
    try:
        from trn_agent_boot.trn_boot import _ntff_profile_via_ctypes

        mod.set_axon_ntff_profile_hook(
            _ntff_profile_via_ctypes("/opt/axon/libaxon_pjrt.so")
        )
    except Exception:
        pass


def _prep_graph(src, dst):
    """Host-side graph preprocessing shared by all layers."""
    import ml_dtypes

    src = np.asarray(src).astype(np.int64).ravel()
    dst = np.asarray(dst).astype(np.int64).ravel()

    deg_src = np.bincount(src, minlength=N_NODES).astype(np.float64)
    deg_dst = np.bincount(dst, minlength=N_NODES).astype(np.float64)
    norm_src = np.clip(deg_src, 1.0, None) ** -0.5
    norm_dst = np.clip(deg_dst, 1.0, None) ** -0.5
    inv_norm_dst = np.sqrt(np.clip(deg_dst, 1.0, None))

    core = dst // NPC
    j = dst % NPC
    wg = (j // 128) // WG_WIN
    # src -> (half, half-local row) -> bank + bank-local row
    s_c = src // NPC
    s_j = src % NPC
    in_a = s_j < HALF_A_ROWS
    hrow = np.where(in_a, s_c * HALF_A_ROWS + s_j,
                    s_c * HALF_B_ROWS + (s_j - HALF_A_ROWS))
    bank_off = np.array([d[1] for d in BANK_DEFS], np.int64)
    b = np.where(in_a, np.where(hrow < BANK_DEFS[1][1], 0, 1),
                 np.where(hrow < BANK_DEFS[3][1], 2, 3))
    srcloc = hrow - bank_off[b]
    blk = (core * NWG + wg) * BANKS + b
    key = (blk.astype(np.int64) << 20) | j
    order = np.argsort(key, kind="stable")
    srcloc_s = srcloc[order]
    j_s = j[order]

    n_blk = NCORES * NWG * BANKS
    counts = np.bincount(blk[order], minlength=n_blk).reshape(NCORES, NWG, BANKS)
    starts = np.zeros(n_blk + 1, np.int64)
    np.cumsum(counts.ravel(), out=starts[1:])
    nsub = np.maximum(1, -(-counts.max(axis=0) // 128))  # [NWG, BANKS]

    # per-core padded dloc/loc blocks; dloc pad = -1, loc pad = 0 (row 0 is
    # real finite data so padded gathers cannot inject NaN into the matmul)
    dloc_blocks = {}
    loc_blocks = {}
    for c in range(NCORES):
        for g in range(NWG):
            for bi in range(BANKS):
                gi = (c * NWG + g) * BANKS + bi
                s0, s1 = starts[gi], starts[gi + 1]
                cap = int(nsub[g, bi]) * 128
                dl = np.full(cap, -1, np.int64)
                lo = np.zeros(cap, np.int64)
                n_e = s1 - s0
                assert n_e <= cap, (n_e, cap)
                dl[:n_e] = j_s[s0:s1] - g * (WG_WIN * 128)
                lo[:n_e] = srcloc_s[s0:s1]
                dloc_blocks[(c, g, bi)] = dl
                loc_blocks[(c, g, bi)] = lo

    # shared schedule: per wg, gathers (one per bank, chunked at MAXSUB) and
    # the (subtile, quadrant) matmul pairs with start/stop flags
    sched = []
    icol = 0
    pcol = 0
    for g in range(NWG):
        nwin = min(WG_WIN, WINDOWS - g * WG_WIN)
        bank_pairs = []
        for bi in range(BANKS):
            ns_ = int(nsub[g, bi])
            qsets = [set() for _ in range(ns_)]
            for c in range(NCORES):
                dl = dloc_blocks[(c, g, bi)]
                for s in range(ns_):
                    rows = dl[s * 128:(s + 1) * 128]
                    qs = np.unique(rows[rows >= 0] // 128)
                    qsets[s].update(int(x) for x in qs)
            bank_pairs.append(
                [(s, q) for s in range(ns_) for q in sorted(qsets[s])]
            )
        covered = set(q for bp in bank_pairs for (_, q) in bp)
        for q in range(nwin):
            if q not in covered:
                bank_pairs[0].insert(0, (0, q))
        all_flat = [(bi, s, q) for bi in range(BANKS) for (s, q) in bank_pairs[bi]]
        firsts = {}
        lasts = {}
        for i, (_, _, q) in enumerate(all_flat):
            firsts.setdefault(q, i)
            lasts[q] = i
        gathers = []
        i_flat = 0
        for bi in range(BANKS):
            ns_ = int(nsub[g, bi])
            pl = bank_pairs[bi]
            off = 0
            while off < ns_:
                csub = min(MAXSUB, ns_ - off)
                mm = []
                for (s, q) in pl:
                    if off <= s < off + csub:
                        mm.append((s - off, pcol, q,
                                   firsts[q] == i_flat, lasts[q] == i_flat))
                        pcol += 1
                        i_flat += 1
                nidx = csub * 128
                gathers.append(dict(b=bi, icol0=icol, nidx=nidx, mm=mm,
                                    sub0=off))
                icol += nidx // 16
                off += csub
        sched.append(dict(nwin=nwin, gathers=gathers))
    total_idxcols = icol
    total_pairs = pcol

    # per-core gather indices + one-hot slabs
    ar128 = np.arange(128)
    per_core = []
    for c in range(NCORES):
        idx16 = np.zeros((128, total_idxcols), np.int16)
        s8 = np.zeros((128, total_pairs * 128), ml_dtypes.float8_e4m3)
        for g in range(NWG):
            for gt in sched[g]["gathers"]:
                bi, icol0, nidx, sub0 = gt["b"], gt["icol0"], gt["nidx"], gt["sub0"]
                lo = loc_blocks[(c, g, bi)][sub0 * 128: sub0 * 128 + nidx]
                stripe = lo.reshape(nidx // 16, 16).T.astype(np.int16)
                for st in range(8):
                    idx16[16 * st:16 * st + 16, icol0:icol0 + nidx // 16] = stripe
                dl = dloc_blocks[(c, g, bi)]
                for (s_loc, pc_, q, _, _) in gt["mm"]:
                    rows = dl[(sub0 + s_loc) * 128:(sub0 + s_loc + 1) * 128]
                    rel = rows - q * 128
                    valid = (rel >= 0) & (rel < 128)
                    m = (rel[:, None] == ar128[None, :]) & valid[:, None]
                    s8[:, pc_ * 128:(pc_ + 1) * 128] = m.astype(
                        ml_dtypes.float8_e4m3
                    )
        per_core.append((idx16, s8))

    def node_tile(vec, c):
        full = np.zeros(WINDOWS * 128, np.float32)
        full[:NPC] = vec[c * NPC:(c + 1) * NPC].astype(np.float32)
        return full.reshape(WINDOWS, 128).T.copy()

    ns_tiles = [node_tile(norm_src, c) for c in range(NCORES)]
    ndn_tiles = [node_tile(norm_dst, c) for c in range(NCORES)]
    invndn_tiles = []
    for c in range(NCORES):
        t = np.zeros((16, WINDOWS * 128), np.float16)
        t[0, :NPC] = inv_norm_dst[c * NPC:(c + 1) * NPC].astype(np.float16)
        invndn_tiles.append(t)

    return sched, total_idxcols, total_pairs, per_core, ns_tiles, ndn_tiles, invndn_tiles


def _build_program(sched, total_idxcols, total_pairs):
    import os

    import concourse.bacc as bacc
    import concourse.mybir as mybir
    import concourse.tile as tile

    dbg_layers = int(os.environ.get("DBG_LAYERS", "4"))

    nc = bacc.Bacc(
        "TRN2",
        target_bir_lowering=False,
        debug=False,
        enable_asserts=False,
        num_devices=NCORES,
        num_swdge_queues=NQ,
    )
    f32, f16, i16 = mybir.dt.float32, mybir.dt.float16, mybir.dt.int16
    f8 = mybir.dt.float8e4

    x_in = nc.dram_tensor("x", [NPC, D], f32, kind="ExternalInput")
    idx_in = nc.dram_tensor("idx16", [128, total_idxcols], i16, kind="ExternalInput")
    s8_in = nc.dram_tensor("s8", [128, total_pairs * D], f8, kind="ExternalInput")
    ns_in = nc.dram_tensor("ns", [128, WINDOWS], f32, kind="ExternalInput")
    ndn_in = nc.dram_tensor("ndn", [128, WINDOWS], f32, kind="ExternalInput")
    invndn_in = nc.dram_tensor("invndn", [16, WINDOWS * 128], f16, kind="ExternalInput")
    w_in = [nc.dram_tensor(f"W{i+1}", [D, D], f16, kind="ExternalInput") for i in range(4)]
    brow_in = [nc.dram_tensor(f"brow{i+1}", [16, D], f16, kind="ExternalInput") for i in range(4)]
    gam_in = nc.dram_tensor("gamma_b", [128, D], f32, kind="ExternalInput")
    bet_in = nc.dram_tensor("beta_b", [128, D], f32, kind="ExternalInput")
    out = nc.dram_tensor("out", [NPC, D], f32, kind="ExternalOutput")

    Gelu = mybir.ActivationFunctionType.Gelu
    Sqrt = mybir.ActivationFunctionType.Sqrt
    Copy = mybir.ActivationFunctionType.Copy
    Ident = mybir.ActivationFunctionType.Identity
    Square = mybir.ActivationFunctionType.Square
    MUL = mybir.AluOpType.mult
    ADD = mybir.AluOpType.add

    qcnt = [0]

    with tile.TileContext(nc) as tc:
        with (
            tc.tile_pool(name="const", bufs=1) as constp,
            tc.tile_pool(name="meta", bufs=1) as metap,
            tc.tile_pool(name="xp", bufs=3) as xp,
            tc.tile_pool(name="msgp", bufs=(24 if MAXSUB <= 10 else 10)) as msgp,
            tc.tile_pool(name="sp", bufs=(24 if MAXSUB <= 10 else 10)) as sp,
            tc.tile_pool(name="aggp", bufs=6) as aggp,
            tc.tile_pool(name="hp", bufs=6) as hp,
            tc.tile_pool(name="lnp", bufs=4) as lnp,
            tc.tile_pool(name="ps1", bufs=3, space="PSUM") as ps1,
            tc.tile_pool(name="ps2", bufs=4, space="PSUM") as ps2,
            tc.tile_pool(name="dram", bufs=1, space="DRAM") as dram,
        ):
            # ---- constants / metadata into SBUF ----
            idx_sb = metap.tile([128, total_idxcols], i16)
            nc.sync.dma_start(idx_sb[:], idx_in[:])
            ns_sb = constp.tile([128, WINDOWS], f32)
            nc.sync.dma_start(ns_sb[:], ns_in[:])
            ndn_sb = constp.tile([128, WINDOWS], f32)
            nc.sync.dma_start(ndn_sb[:], ndn_in[:])
            invndn_sb = constp.tile([16, WINDOWS * 128], f16)
            nc.sync.dma_start(invndn_sb[:], invndn_in[:])
            gam_sb = constp.tile([128, D], f32)
            nc.sync.dma_start(gam_sb[:], gam_in[:])
            bet_sb = constp.tile([128, D], f32)
            nc.sync.dma_start(bet_sb[:], bet_in[:])
            w_sb = []
            brow_sb = []
            for i in range(4):
                wt = constp.tile([D, D], f16, name=f"w{i}_sb")
                nc.sync.dma_start(wt[:], w_in[i][:])
                w_sb.append(wt)
                bt = constp.tile([16, D], f16, name=f"brow{i}_sb")
                nc.sync.dma_start(bt[:], brow_in[i][:])
                brow_sb.append(bt)
            eps_t = constp.tile([128, 1], f32)
            nc.vector.memset(eps_t[:], 1e-5)

            # ---- DRAM h buffers (split halves for chunked AllGather) ----
            h_shA = [
                dram.tile([HALF_A_ROWS, D], f16, name=f"h_shA{l}") for l in range(4)
            ]
            h_shB = [
                dram.tile([HALF_B_ROWS, D], f16, name=f"h_shB{l}") for l in range(4)
            ]
            h_fA = [
                dram.tile([HA_TOT, D], f16, addr_space="Shared", name=f"h_fA{l}")
                for l in range(4)
            ]
            h_fB = [
                dram.tile([HB_TOT, D], f16, addr_space="Shared", name=f"h_fB{l}")
                for l in range(4)
            ]
            rg = [list(range(NCORES))]

            def store_h(l, w, rows, h16):
                if w < 48:
                    nc.sync.dma_start(
                        h_shA[l][w * 128:w * 128 + rows, :], h16[:rows])
                else:
                    r0 = (w - 48) * 128
                    nc.sync.dma_start(
                        h_shB[l][r0:r0 + rows, :], h16[:rows])

            def ag(l, half):
                sh, fl = (h_shA, h_fA) if half == 0 else (h_shB, h_fB)
                nc.gpsimd.collective_compute(
                    "AllGather", mybir.AluOpType.bypass, replica_groups=rg,
                    ins=[sh[l][:]], outs=[fl[l][:]],
                )

            # ---- prologue: h0 = x * norm_src (cast fp16) ----
            for w in range(WINDOWS):
                rows = min(128, NPC - w * 128)
                xt = xp.tile([128, D], f32, tag="xt")
                nc.sync.dma_start(xt[:rows], x_in[w * 128:w * 128 + rows, :])
                ht = xp.tile([128, D], f16, tag="ht0")
                nc.scalar.activation(out=ht[:], in_=xt[:], func=Copy,
                                     scale=ns_sb[:, w:w + 1])
                store_h(0, w, rows, ht)
                if w == 47:
                    ag(0, 0)
            ag(0, 1)

            # ---- layers ----
            for l in range(dbg_layers):
                for g in range(NWG):
                    nwin = sched[g]["nwin"]
                    psg = ps1.tile([128, WG_WIN * 128], f32, tag="psg")
                    qpairs = [[] for _ in range(nwin)]
                    for gt in sched[g]["gathers"]:
                        bi, icol0, nidx = gt["b"], gt["icol0"], gt["nidx"]
                        mm = gt["mm"]
                        if not mm:
                            continue
                        csub = nidx // 128
                        half, roff, rcnt = BANK_DEFS[bi]
                        h_in = (h_fA if half == 0 else h_fB)[l]
                        msg = msgp.tile([128, csub * D], f16, tag="msg")
                        nc.gpsimd.dma_gather(
                            msg[:].rearrange("p (k d) -> p k d", d=D),
                            h_in[roff:roff + rcnt, :],
                            idx_sb[:, icol0:icol0 + nidx // 16],
                            nidx, nidx, D,
                            queue_num=qcnt[0] % NQ,
                        )
                        qcnt[0] += 1
                        np_ = len(mm)
                        pc0 = mm[0][1]
                        s_run = sp.tile([128, np_ * D], f8, tag="s")
                        nc.sync.dma_start(
                            s_run[:],
                            s8_in[:, pc0 * D:(pc0 + np_) * D],
                        )
                        for (s_loc, pc_, q, _, _) in mm:
                            qpairs[q].append((msg, s_run, pc0, s_loc, pc_))
                    # contiguous start->stop accumulation group per quadrant
                    for q in range(nwin):
                        n_q = len(qpairs[q])
                        for i, (msg, s_run, pc0, s_loc, pc_) in enumerate(
                                qpairs[q]):
                            nc.tensor.matmul(
                                psg[:, q * 128:(q + 1) * 128],
                                lhsT=msg[:, s_loc * D:(s_loc + 1) * D],
                                rhs=s_run[:, (pc_ - pc0) * D:(pc_ - pc0 + 1) * D],
                                start=(i == 0), stop=(i == n_q - 1),
                            )
                    for qi in range(nwin):
                        w = g * WG_WIN + qi
                        rows = min(128, NPC - w * 128)
                        aggT = aggp.tile([128, 128], f16, tag="aggT")
                        nc.scalar.copy(out=aggT[:],
                                       in_=psg[:, qi * 128:(qi + 1) * 128])
                        psum2 = ps2.tile([128, 128], f32, tag="psum2")
                        nc.tensor.matmul(psum2[:], lhsT=aggT[:], rhs=w_sb[l][:],
                                         start=True, stop=False)
                        nc.tensor.matmul(
                            psum2[:],
                            lhsT=invndn_sb[:, w * 128:(w + 1) * 128],
                            rhs=brow_sb[l][:],
                            start=False, stop=True,
                        )
                        if l < dbg_layers - 1:
                            g32 = hp.tile([128, D], f32, tag="g32")
                            nc.scalar.activation(out=g32[:], in_=psum2[:],
                                                 func=Gelu,
                                                 scale=ndn_sb[:, w:w + 1])
                            h16 = hp.tile([128, D], f16, tag="h16")
                            nc.scalar.activation(out=h16[:], in_=g32[:],
                                                 func=Copy,
                                                 scale=ns_sb[:, w:w + 1])
                            store_h(l + 1, w, rows, h16)
                        else:
                            # LayerNorm over features (scalar engine + accum)
                            t2 = lnp.tile([128, D], f32, tag="t2")
                            s1 = lnp.tile([128, 1], f32, tag="s1")
                            nc.scalar.activation(out=t2[:], in_=psum2[:],
                                                 func=Copy,
                                                 scale=ndn_sb[:, w:w + 1],
                                                 accum_out=s1[:])
                            negmu = lnp.tile([128, 1], f32, tag="negmu")
                            nc.scalar.mul(out=negmu[:], in_=s1[:], mul=-1.0 / D)
                            cent = lnp.tile([128, D], f32, tag="cent")
                            nc.scalar.activation(out=cent[:], in_=t2[:],
                                                 func=Ident, bias=negmu[:])
                            sq = lnp.tile([128, D], f32, tag="sq")
                            vs = lnp.tile([128, 1], f32, tag="vs")
                            nc.scalar.activation(out=sq[:], in_=cent[:],
                                                 func=Square, accum_out=vs[:])
                            std = lnp.tile([128, 1], f32, tag="std")
                            nc.scalar.activation(out=std[:], in_=vs[:],
                                                 func=Sqrt, scale=1.0 / D,
                                                 bias=eps_t[:])
                            rstd = lnp.tile([128, 1], f32, tag="rstd")
                            nc.vector.reciprocal(out=rstd[:], in_=std[:])
                            t1 = lnp.tile([128, D], f32, tag="t1")
                            nc.scalar.activation(out=t1[:], in_=cent[:],
                                                 func=Copy, scale=rstd[:])
                            t4 = lnp.tile([128, D], f32, tag="t4")
                            nc.vector.tensor_tensor(out=t4[:], in0=t1[:],
                                                    in1=gam_sb[:], op=MUL)
                            t5 = lnp.tile([128, D], f32, tag="t5")
                            nc.vector.tensor_tensor(out=t5[:], in0=t4[:],
                                                    in1=bet_sb[:], op=ADD)
                            nc.sync.dma_start(
                                out[w * 128:w * 128 + rows, :], t5[:rows]
                            )
                    if l < dbg_layers - 1 and g == WG_SPLIT - 1:
                        ag(l + 1, 0)
                if l < dbg_layers - 1:
                    ag(l + 1, 1)
    nc.compile()
    return nc


def kernel(**inputs):
    global LAST_EXEC_NS
    from concourse.bass_utils import run_bass_kernel_spmd

    x = np.asarray(inputs["x"], np.float32)
    src = inputs["src"]
    dst = inputs["dst"]

    key = "prog"
    if key not in _CACHE:
        sched, tic, tpc, per_core, ns_tiles, ndn_tiles, invndn_tiles = _prep_graph(src, dst)
        nc = _build_program(sched, tic, tpc)
        _CACHE[key] = (nc, per_core, ns_tiles, ndn_tiles, invndn_tiles)
    nc, per_core, ns_tiles, ndn_tiles, invndn_tiles = _CACHE[key]

    gamma = np.asarray(inputs["gamma"], np.float32).reshape(1, D)
    beta = np.asarray(inputs["beta"], np.float32).reshape(1, D)
    gamma_b = np.repeat(gamma, 128, axis=0)
    beta_b = np.repeat(beta, 128, axis=0)

    in_maps = []
    for c in range(NCORES):
        idx16, s8 = per_core[c]
        m = {
            "x": np.ascontiguousarray(x[c * NPC:(c + 1) * NPC]),
            "idx16": idx16,
            "s8": s8,
            "ns": ns_tiles[c],
            "ndn": ndn_tiles[c],
            "invndn": invndn_tiles[c],
            "gamma_b": gamma_b,
            "beta_b": beta_b,
        }
        for i in range(4):
            m[f"W{i+1}"] = np.asarray(inputs[f"W{i+1}"], np.float32).astype(np.float16)
            br = np.zeros((16, D), np.float16)
            br[0] = np.asarray(inputs[f"b{i+1}"], np.float32).astype(np.float16)
            m[f"brow{i+1}"] = br
        in_maps.append(m)

    if TRACE:
        _install_ntff_hook()
    res = run_bass_kernel_spmd(
        nc, in_maps, core_ids=list(range(NCORES)), trace=TRACE
    )
    LAST_EXEC_NS = res.exec_time_ns
    return np.concatenate(
        [res.results[c]["out"] for c in range(NCORES)], axis=0
    ).astype(np.float32)
